# revision 12
# baseline (speedup 1.0000x reference)
"""EGNN layer (fully-connected graph, N=384, H=256) on 8 TRN2 NeuronCores.

Sharding: receivers are split 48 per core. Each core computes, for its 48
receivers i and all 384 senders j (self-edge included, corrected exactly):

  h1(j,i)  = silu( eW1a@n_i + eW1b@n_j + w1c*radial(i,j) + b1 )
  msg(j,i) = silu( eW2@h1 + b2 )
  agg(i)   = sum_j msg(j,i) - msg(i,i)
  nodes'   = node MLP + residual (for the 48 local nodes)
  scale(j,i) -> S[i,j];  pos update factored as
  sum_i (p_j - p_i) S[i,j] = p_j * colsum(S) - (P^T S)  (clip never binds)

The radial term is folded into the tensor engine:
  w1c_k * radial(i,j) = w1c_k q_i (per-receiver bias) + w1c_k q_j (folded
  into the sender tables) - 2 w1c_k <p_i, p_j> (a K=3 matmul per receiver).

All matmuls run in float32r (full fp32 data, bf16-rate on the PE for
moving dim >= 256). Everything stays on-chip; only per-core outputs
[256,48] nodes and the [4,384] S-products leave; host does the final
concat / 8-way reduction (pure unsharding).
"""

import sys
import os

sys.path.insert(0, "/opt/trn_rl_repo")

import numpy as np

import concourse.bass as bass
import concourse.mybir as mybir
import concourse.tile as tile
from concourse.bass import ds
from concourse.bass_utils import run_bass_kernel_spmd

F32 = mybir.dt.float32
F32R = mybir.dt.float32r
ACTF = mybir.ActivationFunctionType
ALU = mybir.AluOpType
AX = mybir.AxisListType

N = 384          # nodes
H = 256          # hidden
NCORES = 8
NB = N // NCORES  # receivers per core (48)
G = 6            # receivers per ACT batch group
NGRP = NB // G   # 8 groups
P = 128          # partitions


def _split_waits(nc, max_waits=1):
    """walrus in this container allows 1 inline sync-wait per instruction;
    move extra waits onto same-engine drain carriers inserted just before."""
    n = 0
    for f in nc.m.functions:
        for blk in f.blocks:
            out = []
            for inst in blk.instructions:
                si = getattr(inst, "sync_info", None)
                if si is not None and si.on_wait and len(si.on_wait) > max_waits:
                    waits = list(si.on_wait)
                    extra, keep = waits[:-max_waits], waits[-max_waits:]
                    for j, w in enumerate(extra):
                        d = mybir.InstDrain(
                            name=f"{inst.name}-wsplit{j}", ins=[], outs=[])
                        d.engine = inst.engine
                        d.sync_info = mybir.SyncInfo(on_wait=[w], on_update=[])
                        out.append(d)
                    inst.sync_info = mybir.SyncInfo(
                        on_wait=keep, on_update=list(si.on_update or []))
                    n += 1
                out.append(inst)
            blk.instructions = out
    return n


def build():
    nc = bass.Bass()
    dp = nc.declare_dram_parameter

    # ---- inputs (per-core; host marshals layouts) ----
    ndT_e = dp("ndT", [H, N], F32, isOutput=False)        # nodes.T (replicated)
    posT_e = dp("posT", [3, N], F32, isOutput=False)      # pos.T
    ndrT_e = dp("ndrT", [H, NB], F32, isOutput=False)     # local nodes.T
    posrT_e = dp("posrT", [3, NB], F32, isOutput=False)   # local pos.T
    w1aT_e = dp("w1aT", [H, H], F32, isOutput=False)      # eW1[:, :256].T
    w1bT_e = dp("w1bT", [H, H], F32, isOutput=False)      # eW1[:, 256:512].T
    w1c_e = dp("w1c", [1, H], F32, isOutput=False)        # eW1[:, 512]
    w2T_e = dp("w2T", [H, H], F32, isOutput=False)        # eW2.T
    pw1T_e = dp("pw1T", [H, H], F32, isOutput=False)      # pW1.T
    pw2r_e = dp("pw2row", [1, H], F32, isOutput=False)    # pW2 row
    pr4_e = dp("posr4rows", [1, 4 * NB], F32, isOutput=False)  # [p_i|1] rows
    nw1T_e = dp("nw1T", [2 * H, H], F32, isOutput=False)  # nW1.T
    nw2T_e = dp("nw2T", [H, H], F32, isOutput=False)      # nW2.T
    eb1_e = dp("eb1", [H, 1], F32, isOutput=False)
    eb2_e = dp("eb2", [H, 1], F32, isOutput=False)
    pb1_e = dp("pb1", [H, 1], F32, isOutput=False)
    nb1_e = dp("nb1", [H, 1], F32, isOutput=False)
    nb2_e = dp("nb2", [H, 1], F32, isOutput=False)

    # ---- outputs ----
    noutT_e = dp("nodes_outT", [H, NB], F32, isOutput=True)
    pout_e = dp("pos_out", [4, N], F32, isOutput=True)

    with tile.TileContext(nc) as tc:
        with tc.tile_pool(name="const", bufs=1) as cp, \
             tc.tile_pool(name="stage", bufs=2) as stp, \
             tc.tile_pool(name="l3p", bufs=3) as l3p, \
             tc.tile_pool(name="psum", bufs=2, space="PSUM") as pp:

            dma = nc.sync.dma_start

            # ============ loads ============
            ndT_f = [cp.tile([P, N], F32, tag=f"ndT{h}", name=f"ndT{h}") for h in range(2)]
            for h in range(2):
                dma(out=ndT_f[h], in_=ndT_e[ds(P * h, P), :])
            posT_f = cp.tile([3, N], F32, tag="posT", name="posT")
            dma(out=posT_f, in_=posT_e[:, :])
            ndrT_f = [cp.tile([P, NB], F32, tag=f"ndrT{h}", name=f"ndrT{h}") for h in range(2)]
            for h in range(2):
                dma(out=ndrT_f[h], in_=ndrT_e[ds(P * h, P), :])
            posrT_f = cp.tile([3, NB], F32, tag="posrT", name="posrT")
            dma(out=posrT_f, in_=posrT_e[:, :])

            def load_w(ext, rows, name):
                nkc = rows // P
                tiles = [cp.tile([P, ext.shape[1]], F32, tag=f"{name}{kc}", name=f"{name}{kc}")
                         for kc in range(nkc)]
                for kc in range(nkc):
                    dma(out=tiles[kc], in_=ext[ds(P * kc, P), :])
                return tiles

            w1aT_f = load_w(w1aT_e, H, "w1aT")
            w1bT_f = load_w(w1bT_e, H, "w1bT")
            w2T_f = load_w(w2T_e, H, "w2T")
            pw1T_f = load_w(pw1T_e, H, "pw1T")
            nw1T_f = load_w(nw1T_e, 2 * H, "nw1T")
            nw2T_f = load_w(nw2T_e, H, "nw2T")
            pw2r_f = cp.tile([1, H], F32, tag="pw2row", name="pw2row")
            dma(out=pw2r_f, in_=pw2r_e[:, :])
            pr4_f = cp.tile([1, 4 * NB], F32, tag="pr4", name="pr4")
            dma(out=pr4_f, in_=pr4_e[:, :])
            w1c_f = cp.tile([1, H], F32, tag="w1c", name="w1c")
            dma(out=w1c_f, in_=w1c_e[:, :])

            def load_bias(ext, name, rows=H):
                tiles = [cp.tile([P, 1], F32, tag=f"{name}{h}", name=f"{name}{h}")
                         for h in range(rows // P)]
                for h in range(rows // P):
                    dma(out=tiles[h], in_=ext[ds(P * h, P), :])
                return tiles

            eb1c = load_bias(eb1_e, "eb1")
            eb2c = load_bias(eb2_e, "eb2")
            pb1c = load_bias(pb1_e, "pb1")
            nb1c = load_bias(nb1_e, "nb1")
            nb2c = load_bias(nb2_e, "nb2")

            # ============ f32r working copies (producer must round) ============
            def to_r(src_tiles, name):
                out = []
                for i, s in enumerate(src_tiles):
                    t = cp.tile(list(s.shape), F32R, tag=f"{name}{i}r", name=f"{name}{i}r")
                    nc.vector.tensor_copy(t, s)
                    out.append(t)
                return out

            ndT_r = to_r(ndT_f, "ndT")
            ndrT_r = to_r(ndrT_f, "ndrT")
            posT_r = to_r([posT_f], "posT")[0]
            pw2r_r = to_r([pw2r_f], "pw2row")[0]
            pr4_r = to_r([pr4_f], "pr4")[0]
            w1aT_r = to_r(w1aT_f, "w1aT")
            w1bT_r = to_r(w1bT_f, "w1bT")
            w2T_r = to_r(w2T_f, "w2T")
            pw1T_r = to_r(pw1T_f, "pw1T")
            nw1T_r = to_r(nw1T_f, "nw1T")
            nw2T_r = to_r(nw2T_f, "nw2T")
            w1c_r = to_r([w1c_f], "w1c")[0]
            w1abT_r = []
            for kc in range(2):
                t = cp.tile([P, H], F32R, tag=f"w1abT{kc}r", name=f"w1abT{kc}r")
                nc.vector.tensor_add(t, w1aT_f[kc], w1bT_f[kc])
                w1abT_r.append(t)

            # ============ q = |pos|^2 rows ============
            ones31_f = cp.tile([3, 1], F32, tag="ones31", name="ones31")
            nc.vector.memset(ones31_f, 1.0)
            ones31_r = to_r([ones31_f], "ones31")[0]
            ones13_f = cp.tile([1, 3], F32, tag="ones13", name="ones13")
            nc.vector.memset(ones13_f, 1.0)
            ones13_r = to_r([ones13_f], "ones13")[0]

            sq_r = cp.tile([3, N], F32R, tag="sq", name="sq")
            nc.vector.tensor_mul(sq_r, posT_f, posT_f)
            q_ps = pp.tile([1, N], F32, tag="scl", name="scl", bufs=1)
            nc.tensor.matmul(q_ps, ones31_r, sq_r, start=True, stop=True)
            q_r = cp.tile([1, N], F32R, tag="qrow", name="qrow")
            nc.vector.tensor_copy(q_r, q_ps)

            sqloc_r = cp.tile([3, NB], F32R, tag="sqloc", name="sqloc")
            nc.vector.tensor_mul(sqloc_r, posrT_f, posrT_f)
            qloc_ps = pp.tile([1, NB], F32, tag="scl", name="scl", bufs=1)
            nc.tensor.matmul(qloc_ps, ones31_r, sqloc_r, start=True, stop=True)
            qloc_r = cp.tile([1, NB], F32R, tag="qloc", name="qloc")
            nc.vector.tensor_copy(qloc_r, qloc_ps)

            # w1c broadcast to 3 partitions (for per-receiver cross lhsT)
            w1cb3_ps = pp.tile([3, H], F32, tag="scl", name="scl", bufs=1)
            nc.tensor.matmul(w1cb3_ps, ones13_r, w1c_r, start=True, stop=True)
            w1cb3_f = cp.tile([3, H], F32, tag="w1cb3", name="w1cb3")
            nc.vector.tensor_copy(w1cb3_f, w1cb3_ps)

            # ============ sender tables B'[k,j] = eW1b@n_j + w1c_k q_j ======
            BTp_f = []
            for h in range(2):
                ps = pp.tile([P, N], F32, tag="mm", name="mm")
                nc.tensor.matmul(ps, w1bT_r[0][:, ds(P * h, P)], ndT_r[0],
                                 start=True, stop=False)
                nc.tensor.matmul(ps, w1bT_r[1][:, ds(P * h, P)], ndT_r[1],
                                 start=False, stop=False)
                nc.tensor.matmul(ps, w1c_r[:, ds(P * h, P)], q_r,
                                 start=False, stop=True)
                t = cp.tile([P, N], F32, tag=f"BTp{h}", name=f"BTp{h}")
                nc.vector.tensor_copy(t, ps)
                BTp_f.append(t)

            # ==== receiver bias A'[k,i] = eW1a@n_i + b1 + w1c_k q_i (local) ====
            A2loc_f = []
            for h in range(2):
                ps = pp.tile([P, NB], F32, tag="s1", name="s1")
                nc.tensor.matmul(ps, w1aT_r[0][:, ds(P * h, P)], ndrT_r[0],
                                 start=True, stop=False)
                nc.tensor.matmul(ps, w1aT_r[1][:, ds(P * h, P)], ndrT_r[1],
                                 start=False, stop=False)
                nc.tensor.matmul(ps, w1c_r[:, ds(P * h, P)], qloc_r,
                                 start=False, stop=True)
                t = cp.tile([P, NB], F32, tag=f"A2loc{h}", name=f"A2loc{h}")
                nc.vector.tensor_scalar_add(t, ps, eb1c[h])
                A2loc_f.append(t)

            # ============ self messages msg(i,i) (radial = 0 exactly) ======
            h1self_r = []
            for h in range(2):
                ps = pp.tile([P, NB], F32, tag="s1", name="s1")
                nc.tensor.matmul(ps, w1abT_r[0][:, ds(P * h, P)], ndrT_r[0],
                                 start=True, stop=False)
                nc.tensor.matmul(ps, w1abT_r[1][:, ds(P * h, P)], ndrT_r[1],
                                 start=False, stop=True)
                t = cp.tile([P, NB], F32R, tag=f"h1self{h}", name=f"h1self{h}")
                nc.scalar.activation(t, ps, ACTF.Silu, bias=eb1c[h])
                h1self_r.append(t)
            msgself_f = []
            for h in range(2):
                ps = pp.tile([P, NB], F32, tag="mm", name="mm")
                nc.tensor.matmul(ps, w2T_r[0][:, ds(P * h, P)], h1self_r[0],
                                 start=True, stop=False)
                nc.tensor.matmul(ps, w2T_r[1][:, ds(P * h, P)], h1self_r[1],
                                 start=False, stop=True)
                t = cp.tile([P, NB], F32, tag=f"msgself{h}", name=f"msgself{h}")
                nc.scalar.activation(t, ps, ACTF.Silu, bias=eb2c[h])
                msgself_f.append(t)

            # ==== pos-update lhsT: M4[kc][k, 4i:4i+4] = pW2[k]*[p_i|1] ====
            M4_r = []
            for kc in range(2):
                ps = pp.tile([P, 4 * NB], F32, tag="scl", name="m4ps", bufs=1)
                for i in range(NB):
                    nc.tensor.matmul(ps[:, ds(4 * i, 4)],
                                     pw2r_r[:, ds(P * kc, P)],
                                     pr4_r[:, ds(4 * i, 4)],
                                     start=True, stop=True)
                t = cp.tile([P, 4 * NB], F32R, tag=f"M4_{kc}", name=f"M4_{kc}")
                nc.vector.tensor_copy(t, ps)
                M4_r.append(t)
            p4ps = pp.tile([4, N], F32, tag="p4", name="p4ps", bufs=1)

            # ============ edge pipeline ============
            aggT_f = [cp.tile([P, NB], F32, tag=f"aggT{h}", name=f"aggT{h}") for h in range(2)]

            for g in range(NGRP):
                h1stg = [stp.tile([P, G * N], F32R, tag=f"h1stg{h}", name=f"h1stg{h}")
                         for h in range(2)]
                msgstg = [stp.tile([P, G * N], F32R, tag=f"msgstg{h}", name=f"msgstg{h}")
                          for h in range(2)]
                p1stg = [stp.tile([P, G * N], F32R, tag=f"p1stg{h}", name=f"p1stg{h}")
                         for h in range(2)]

                # stage 1: h1_pre = cross(K=3 matmul) + A'col + B'
                for i in range(G):
                    iloc = G * g + i
                    l3 = l3p.tile([3, H], F32R, tag="lhsT3", name="lhsT3")
                    nc.vector.tensor_scalar(
                        out=l3, in0=w1cb3_f,
                        scalar1=posrT_f[:, ds(iloc, 1)], scalar2=-2.0,
                        op0=ALU.mult, op1=ALU.mult)
                    for h in range(2):
                        ps = pp.tile([P, N], F32, tag="s1", name="s1")
                        nc.tensor.matmul(ps, l3[:, ds(P * h, P)], posT_r,
                                         start=True, stop=True)
                        nc.vector.scalar_tensor_tensor(
                            out=h1stg[h][:, ds(i * N, N)], in0=ps,
                            scalar=A2loc_f[h][:, ds(iloc, 1)], in1=BTp_f[h],
                            op0=ALU.add, op1=ALU.add)
                for h in range(2):
                    nc.scalar.activation(h1stg[h], h1stg[h], ACTF.Silu)

                # stage 2: msg = silu(eW2 @ h1 + b2)
                for i in range(G):
                    for h in range(2):
                        ps = pp.tile([P, N], F32, tag="mm", name="mm")
                        nc.tensor.matmul(ps, w2T_r[0][:, ds(P * h, P)],
                                         h1stg[0][:, ds(i * N, N)],
                                         start=True, stop=False)
                        nc.tensor.matmul(ps, w2T_r[1][:, ds(P * h, P)],
                                         h1stg[1][:, ds(i * N, N)],
                                         start=False, stop=True)
                        nc.vector.tensor_copy(msgstg[h][:, ds(i * N, N)], ps)
                for h in range(2):
                    nc.scalar.activation(msgstg[h], msgstg[h], ACTF.Silu,
                                         bias=eb2c[h])

                # stage 3: p1 = silu(pW1 @ msg + pb1); agg by receiver
                for i in range(G):
                    iloc = G * g + i
                    for h in range(2):
                        ps = pp.tile([P, N], F32, tag="p1", name="p1")
                        nc.tensor.matmul(ps, pw1T_r[0][:, ds(P * h, P)],
                                         msgstg[0][:, ds(i * N, N)],
                                         start=True, stop=False)
                        nc.tensor.matmul(ps, pw1T_r[1][:, ds(P * h, P)],
                                         msgstg[1][:, ds(i * N, N)],
                                         start=False, stop=True)
                        nc.vector.tensor_copy(p1stg[h][:, ds(i * N, N)], ps)
                        nc.vector.tensor_reduce(
                            aggT_f[h][:, ds(iloc, 1)],
                            msgstg[h][:, ds(i * N, N)], AX.X, ALU.add)
                        nc.vector.tensor_sub(
                            aggT_f[h][:, ds(iloc, 1)],
                            aggT_f[h][:, ds(iloc, 1)],
                            msgself_f[h][:, ds(iloc, 1)])
                for h in range(2):
                    nc.scalar.activation(p1stg[h], p1stg[h], ACTF.Silu,
                                         bias=pb1c[h])

                # stage 4: psum4 += M4_i.T @ p1_i  (pb2 handled on host)
                for i in range(G):
                    iloc = G * g + i
                    first = (iloc == 0)
                    last = (iloc == NB - 1)
                    nc.tensor.matmul(p4ps, M4_r[0][:, ds(4 * iloc, 4)],
                                     p1stg[0][:, ds(i * N, N)],
                                     start=first, stop=False,
                                     skip_group_check=True)
                    nc.tensor.matmul(p4ps, M4_r[1][:, ds(4 * iloc, 4)],
                                     p1stg[1][:, ds(i * N, N)],
                                     start=False, stop=last,
                                     skip_group_check=True)

            # ============ node MLP + residual ============
            aggT_r = to_r(aggT_f, "aggT")
            nh_r = []
            for h in range(2):
                ps = pp.tile([P, NB], F32, tag="s1", name="s1")
                nc.tensor.matmul(ps, nw1T_r[0][:, ds(P * h, P)], ndrT_r[0],
                                 start=True, stop=False)
                nc.tensor.matmul(ps, nw1T_r[1][:, ds(P * h, P)], ndrT_r[1],
                                 start=False, stop=False)
                nc.tensor.matmul(ps, nw1T_r[2][:, ds(P * h, P)], aggT_r[0],
                                 start=False, stop=False)
                nc.tensor.matmul(ps, nw1T_r[3][:, ds(P * h, P)], aggT_r[1],
                                 start=False, stop=True)
                t = cp.tile([P, NB], F32R, tag=f"nh{h}", name=f"nh{h}")
                nc.scalar.activation(t, ps, ACTF.Silu, bias=nb1c[h])
                nh_r.append(t)
            for h in range(2):
                ps = pp.tile([P, NB], F32, tag="mm", name="mm")
                nc.tensor.matmul(ps, nw2T_r[0][:, ds(P * h, P)], nh_r[0],
                                 start=True, stop=False)
                nc.tensor.matmul(ps, nw2T_r[1][:, ds(P * h, P)], nh_r[1],
                                 start=False, stop=True)
                t = cp.tile([P, NB], F32, tag=f"noutT{h}", name=f"noutT{h}")
                nc.vector.scalar_tensor_tensor(
                    out=t, in0=ps, scalar=nb2c[h], in1=ndrT_f[h],
                    op0=ALU.add, op1=ALU.add)
                dma(out=noutT_e[ds(P * h, P), :], in_=t)

            # ============ pos products out ============
            pout_f = cp.tile([4, N], F32, tag="pout", name="pout")
            nc.vector.tensor_copy(pout_f, p4ps)
            dma(out=pout_e[:, :], in_=pout_f)

    _split_waits(nc)
    return nc


_NC_CACHE = None


def _get_nc():
    global _NC_CACHE
    if _NC_CACHE is None:
        _NC_CACHE = build()
    return _NC_CACHE


def kernel(**inputs):
    f32 = np.float32
    nodes = np.ascontiguousarray(np.asarray(inputs["nodes"], f32))
    pos = np.ascontiguousarray(np.asarray(inputs["pos"], f32))
    eW1 = np.asarray(inputs["eW1"], f32)
    eW2 = np.asarray(inputs["eW2"], f32)
    pW1 = np.asarray(inputs["pW1"], f32)
    pW2 = np.asarray(inputs["pW2"], f32)
    nW1 = np.asarray(inputs["nW1"], f32)
    nW2 = np.asarray(inputs["nW2"], f32)

    def c(x):
        return np.ascontiguousarray(x.astype(f32))

    shared = {
        "ndT": c(nodes.T),
        "posT": c(pos.T),
        "w1aT": c(eW1[:, :H].T),
        "w1bT": c(eW1[:, H:2 * H].T),
        "w1c": c(eW1[:, 2 * H].reshape(1, H)),
        "w2T": c(eW2.T),
        "pw1T": c(pW1.T),
        "pw2row": c(pW2.reshape(1, H)),
        "nw1T": c(nW1.T),
        "nw2T": c(nW2.T),
        "eb1": c(np.asarray(inputs["eb1"], f32).reshape(H, 1)),
        "eb2": c(np.asarray(inputs["eb2"], f32).reshape(H, 1)),
        "pb1": c(np.asarray(inputs["pb1"], f32).reshape(H, 1)),
        "nb1": c(np.asarray(inputs["nb1"], f32).reshape(H, 1)),
        "nb2": c(np.asarray(inputs["nb2"], f32).reshape(H, 1)),
    }
    in_maps = []
    for cix in range(NCORES):
        blk = slice(NB * cix, NB * (cix + 1))
        m = dict(shared)
        m["ndrT"] = c(nodes[blk].T)
        m["posrT"] = c(pos[blk].T)
        m["posr4rows"] = c(np.concatenate(
            [pos[blk], np.ones((NB, 1), f32)], axis=1).reshape(1, 4 * NB))
        in_maps.append(m)

    res = run_bass_kernel_spmd(_get_nc(), in_maps, list(range(NCORES))).results

    new_nodes = np.concatenate(
        [res[cix]["nodes_outT"].T for cix in range(NCORES)], axis=0)

    upd = np.zeros((N, 3), np.float64)
    pos64 = pos.astype(np.float64)
    for cix in range(NCORES):
        p4 = res[cix]["pos_out"].astype(np.float64)
        upd += pos64 * p4[3][:, None] - p4[0:3].T
    # pb2 enters every scale entry; its pos contribution is linear and exact:
    # sum_{i != j} (p_j - p_i) * pb2 = pb2 * (N * p_j - sum_i p_i)
    pb2 = float(np.asarray(inputs["pb2"]).reshape(-1)[0])
    upd += pb2 * (N * pos64 - pos64.sum(axis=0, keepdims=True))
    new_pos = (pos64 + upd).astype(f32)
    return new_nodes, new_pos


# revision 15
# speedup vs baseline: 1.1887x; 1.1887x over previous
"""EGNN layer (fully-connected graph, N=384, H=256) on 8 TRN2 NeuronCores.

Sharding: receivers are split 48 per core. Each core computes, for its 48
receivers i and all 384 senders j (self-edge included, corrected exactly):

  h1(j,i)  = silu( eW1a@n_i + eW1b@n_j + w1c*radial(i,j) + b1 )
  msg(j,i) = silu( eW2@h1 + b2 )
  agg(i)   = sum_j msg(j,i) - msg(i,i)
  nodes'   = node MLP + residual (for the 48 local nodes)
  p1(j,i)  = silu( pW1@msg + pb1 );  psum4 += (pW2*[p_i|1]).T @ p1
  pos update: sum_i (p_j - p_i)*scale = p_j*colsum - P^T S (clip never binds;
  pb2's linear contribution is added exactly on the host)

The radial term is folded into the tensor engine:
  w1c_k * radial(i,j) = w1c_k q_i (receiver bias via scalar op) + w1c_k q_j
  (folded into the sender tables B') - 2 w1c_k <p_i, p_j> (K=3 matmul).

Hot-path matmuls (per-edge stages) run in bf16 (fp32-accumulated); the
one-time precompute (sender tables, receiver biases, node MLP, pos
products) runs in float32r. Only [256,48] nodes and [4,384] pos products
leave each core; the host does the final concat / 8-way reduction.
"""

import sys

sys.path.insert(0, "/opt/trn_rl_repo")

import numpy as np

import concourse.bass as bass
import concourse.mybir as mybir
import concourse.tile as tile
from concourse.bass import ds
from concourse.bass_utils import run_bass_kernel_spmd

F32 = mybir.dt.float32
F32R = mybir.dt.float32r
BF16 = mybir.dt.bfloat16
ACTF = mybir.ActivationFunctionType
ALU = mybir.AluOpType
AX = mybir.AxisListType

N = 384          # nodes
H = 256          # hidden
NCORES = 8
NB = N // NCORES  # receivers per core (48)
G = 8            # receivers per ACT batch group
NGRP = NB // G
P = 128          # partitions


def _split_waits(nc, max_waits=1):
    """walrus in this container allows 1 inline sync-wait per instruction;
    move extra waits onto same-engine NoOp carriers inserted just before."""
    n = 0
    for f in nc.m.functions:
        for blk in f.blocks:
            out = []
            for inst in blk.instructions:
                si = getattr(inst, "sync_info", None)
                if si is not None and si.on_wait and len(si.on_wait) > max_waits:
                    waits = list(si.on_wait)
                    extra, keep = waits[:-max_waits], waits[-max_waits:]
                    for j, w in enumerate(extra):
                        d = mybir.InstNoOp(
                            name=f"{inst.name}-wsplit{j}", ins=[], outs=[])
                        d.engine = inst.engine
                        d.sync_info = mybir.SyncInfo(on_wait=[w], on_update=[])
                        out.append(d)
                    inst.sync_info = mybir.SyncInfo(
                        on_wait=keep, on_update=list(si.on_update or []))
                    n += 1
                out.append(inst)
            blk.instructions = out
    return n


def build():
    nc = bass.Bass()
    dp = nc.declare_dram_parameter

    # ---- inputs (per-core; host marshals layouts) ----
    ndT_e = dp("ndT", [H, N], F32, isOutput=False)        # nodes.T (replicated)
    posT_e = dp("posT", [3, N], F32, isOutput=False)      # pos.T
    ndrT_e = dp("ndrT", [H, NB], F32, isOutput=False)     # local nodes.T
    posrT_e = dp("posrT", [3, NB], F32, isOutput=False)   # local pos.T
    w1aT_e = dp("w1aT", [H, H], F32, isOutput=False)      # eW1[:, :256].T
    w1bT_e = dp("w1bT", [H, H], F32, isOutput=False)      # eW1[:, 256:512].T
    w1c_e = dp("w1c", [1, H], F32, isOutput=False)        # eW1[:, 512]
    w2T_e = dp("w2T", [H, H], F32, isOutput=False)        # eW2.T
    pw1T_e = dp("pw1T", [H, H], F32, isOutput=False)      # pW1.T
    pw2r_e = dp("pw2row", [1, H], F32, isOutput=False)    # pW2 row
    pr4_e = dp("posr4rows", [1, 4 * NB], F32, isOutput=False)  # [p_i|1] rows
    nw1T_e = dp("nw1T", [2 * H, H], F32, isOutput=False)  # nW1.T
    nw2T_e = dp("nw2T", [H, H], F32, isOutput=False)      # nW2.T
    eb1_e = dp("eb1", [H, 1], F32, isOutput=False)
    eb2_e = dp("eb2", [H, 1], F32, isOutput=False)
    pb1_e = dp("pb1", [H, 1], F32, isOutput=False)
    nb1_e = dp("nb1", [H, 1], F32, isOutput=False)
    nb2_e = dp("nb2", [H, 1], F32, isOutput=False)

    # ---- outputs ----
    noutT_e = dp("nodes_outT", [H, NB], F32, isOutput=True)
    pout_e = dp("pos_out", [4, N], F32, isOutput=True)

    with tile.TileContext(nc) as tc:
        with tc.tile_pool(name="const", bufs=1) as cp, \
             tc.tile_pool(name="stage", bufs=2) as stp, \
             tc.tile_pool(name="l3p", bufs=3) as l3p, \
             tc.tile_pool(name="psum", bufs=2, space="PSUM") as pp:

            dma = nc.sync.dma_start

            # ============ loads ============
            ndT_f = [cp.tile([P, N], F32, tag=f"ndT{h}", name=f"ndT{h}")
                     for h in range(2)]
            for h in range(2):
                dma(out=ndT_f[h], in_=ndT_e[ds(P * h, P), :])
            posT_f = cp.tile([3, N], F32, tag="posT", name="posT")
            dma(out=posT_f, in_=posT_e[:, :])
            ndrT_f = [cp.tile([P, NB], F32, tag=f"ndrT{h}", name=f"ndrT{h}")
                      for h in range(2)]
            for h in range(2):
                dma(out=ndrT_f[h], in_=ndrT_e[ds(P * h, P), :])
            posrT_f = cp.tile([3, NB], F32, tag="posrT", name="posrT")
            dma(out=posrT_f, in_=posrT_e[:, :])

            def load_w(ext, rows, name):
                nkc = rows // P
                tiles = [cp.tile([P, ext.shape[1]], F32, tag=f"{name}{kc}",
                                 name=f"{name}{kc}") for kc in range(nkc)]
                for kc in range(nkc):
                    dma(out=tiles[kc], in_=ext[ds(P * kc, P), :])
                return tiles

            w1aT_f = load_w(w1aT_e, H, "w1aT")
            w1bT_f = load_w(w1bT_e, H, "w1bT")
            w2T_f = load_w(w2T_e, H, "w2T")
            pw1T_f = load_w(pw1T_e, H, "pw1T")
            nw1T_f = load_w(nw1T_e, 2 * H, "nw1T")
            nw2T_f = load_w(nw2T_e, H, "nw2T")
            pw2r_f = cp.tile([1, H], F32, tag="pw2row", name="pw2row")
            dma(out=pw2r_f, in_=pw2r_e[:, :])
            pr4_f = cp.tile([1, 4 * NB], F32, tag="pr4", name="pr4")
            dma(out=pr4_f, in_=pr4_e[:, :])
            w1c_f = cp.tile([1, H], F32, tag="w1c", name="w1c")
            dma(out=w1c_f, in_=w1c_e[:, :])

            def load_bias(ext, name, rows=H):
                tiles = [cp.tile([P, 1], F32, tag=f"{name}{h}",
                                 name=f"{name}{h}") for h in range(rows // P)]
                for h in range(rows // P):
                    dma(out=tiles[h], in_=ext[ds(P * h, P), :])
                return tiles

            eb1c = load_bias(eb1_e, "eb1")
            eb2c = load_bias(eb2_e, "eb2")
            pb1c = load_bias(pb1_e, "pb1")
            nb1c = load_bias(nb1_e, "nb1")
            nb2c = load_bias(nb2_e, "nb2")

            # ===== typed working copies (f32r for precompute, bf16 hot) =====
            def to_t(src_tiles, name, dt):
                out = []
                for i, s in enumerate(src_tiles):
                    t = cp.tile(list(s.shape), dt, tag=f"{name}{i}c",
                                name=f"{name}{i}c")
                    nc.vector.tensor_copy(t, s)
                    out.append(t)
                return out

            ndT_r = to_t(ndT_f, "ndT", F32R)
            ndrT_r = to_t(ndrT_f, "ndrT", F32R)
            posT_b = to_t([posT_f], "posTb", BF16)[0]
            pw2r_r = to_t([pw2r_f], "pw2row", F32R)[0]
            pr4_r = to_t([pr4_f], "pr4", F32R)[0]
            w1aT_r = to_t(w1aT_f, "w1aT", F32R)
            w1bT_r = to_t(w1bT_f, "w1bT", F32R)
            w2T_b = to_t(w2T_f, "w2T", BF16)
            pw1T_b = to_t(pw1T_f, "pw1T", BF16)
            nw1T_r = to_t(nw1T_f, "nw1T", F32R)
            nw2T_r = to_t(nw2T_f, "nw2T", F32R)
            w1c_r = to_t([w1c_f], "w1c", F32R)[0]
            w1abT_r = []
            for kc in range(2):
                t = cp.tile([P, H], F32R, tag=f"w1abT{kc}r",
                            name=f"w1abT{kc}r")
                nc.vector.tensor_add(t, w1aT_f[kc], w1bT_f[kc])
                w1abT_r.append(t)

            # ============ q = |pos|^2 rows ============
            ones31_f = cp.tile([3, 1], F32, tag="ones31", name="ones31")
            nc.vector.memset(ones31_f, 1.0)
            ones31_r = to_t([ones31_f], "ones31", F32R)[0]
            ones13_f = cp.tile([1, 3], F32, tag="ones13", name="ones13")
            nc.vector.memset(ones13_f, 1.0)
            ones13_r = to_t([ones13_f], "ones13", F32R)[0]

            sq_r = cp.tile([3, N], F32R, tag="sq", name="sq")
            nc.vector.tensor_mul(sq_r, posT_f, posT_f)
            q_ps = pp.tile([1, N], F32, tag="scl", name="scl", bufs=1)
            nc.tensor.matmul(q_ps, ones31_r, sq_r, start=True, stop=True)
            q_r = cp.tile([1, N], F32R, tag="qrow", name="qrow")
            nc.vector.tensor_copy(q_r, q_ps)

            sqloc_r = cp.tile([3, NB], F32R, tag="sqloc", name="sqloc")
            nc.vector.tensor_mul(sqloc_r, posrT_f, posrT_f)
            qloc_ps = pp.tile([1, NB], F32, tag="scl", name="scl", bufs=1)
            nc.tensor.matmul(qloc_ps, ones31_r, sqloc_r, start=True, stop=True)
            qloc_r = cp.tile([1, NB], F32R, tag="qloc", name="qloc")
            nc.vector.tensor_copy(qloc_r, qloc_ps)

            # w1c broadcast to 3 partitions (for per-receiver cross lhsT)
            w1cb3_ps = pp.tile([3, H], F32, tag="scl", name="scl", bufs=1)
            nc.tensor.matmul(w1cb3_ps, ones13_r, w1c_r, start=True, stop=True)
            w1cb3_b = cp.tile([3, H], BF16, tag="w1cb3", name="w1cb3")
            nc.vector.tensor_copy(w1cb3_b, w1cb3_ps)

            # ============ sender tables B'[k,j] = eW1b@n_j + w1c_k q_j ======
            BTp_f = []
            for h in range(2):
                ps = pp.tile([P, N], F32, tag="s1", name="s1", bufs=1)
                nc.tensor.matmul(ps, w1bT_r[0][:, ds(P * h, P)], ndT_r[0],
                                 start=True, stop=False)
                nc.tensor.matmul(ps, w1bT_r[1][:, ds(P * h, P)], ndT_r[1],
                                 start=False, stop=False)
                nc.tensor.matmul(ps, w1c_r[:, ds(P * h, P)], q_r,
                                 start=False, stop=True)
                t = cp.tile([P, N], F32, tag=f"BTp{h}", name=f"BTp{h}")
                nc.vector.tensor_copy(t, ps)
                BTp_f.append(t)

            # ==== receiver bias A'[k,i] = eW1a@n_i + b1 + w1c_k q_i (local) ====
            A2loc_f = []
            for h in range(2):
                ps = pp.tile([P, NB], F32, tag="s1", name="s1", bufs=1)
                nc.tensor.matmul(ps, w1aT_r[0][:, ds(P * h, P)], ndrT_r[0],
                                 start=True, stop=False)
                nc.tensor.matmul(ps, w1aT_r[1][:, ds(P * h, P)], ndrT_r[1],
                                 start=False, stop=False)
                nc.tensor.matmul(ps, w1c_r[:, ds(P * h, P)], qloc_r,
                                 start=False, stop=True)
                t = cp.tile([P, NB], F32, tag=f"A2loc{h}", name=f"A2loc{h}")
                nc.vector.tensor_scalar_add(t, ps, eb1c[h])
                A2loc_f.append(t)

            # ============ self messages msg(i,i) (radial = 0 exactly) ======
            h1self_b = []
            for h in range(2):
                ps = pp.tile([P, NB], F32, tag="s1", name="s1", bufs=1)
                nc.tensor.matmul(ps, w1abT_r[0][:, ds(P * h, P)], ndrT_r[0],
                                 start=True, stop=False)
                nc.tensor.matmul(ps, w1abT_r[1][:, ds(P * h, P)], ndrT_r[1],
                                 start=False, stop=True)
                t = cp.tile([P, NB], BF16, tag=f"h1self{h}", name=f"h1self{h}")
                nc.scalar.activation(t, ps, ACTF.Silu, bias=eb1c[h])
                h1self_b.append(t)
            msgself_f = []
            for h in range(2):
                ps = pp.tile([P, NB], F32, tag="p1", name="p1")
                nc.tensor.matmul(ps, w2T_b[0][:, ds(P * h, P)], h1self_b[0],
                                 start=True, stop=False)
                nc.tensor.matmul(ps, w2T_b[1][:, ds(P * h, P)], h1self_b[1],
                                 start=False, stop=True)
                t = cp.tile([P, NB], F32, tag=f"msgself{h}", name=f"msgself{h}")
                nc.scalar.activation(t, ps, ACTF.Silu, bias=eb2c[h])
                msgself_f.append(t)

            # ==== pos-update lhsT: M4[kc][k, 4i:4i+4] = pW2[k]*[p_i|1] ====
            M4_b = []
            for kc in range(2):
                ps = pp.tile([P, 4 * NB], F32, tag="scl", name="m4ps", bufs=1)
                nc.tensor.matmul(ps, pw2r_r[:, ds(P * kc, P)], pr4_r,
                                 start=True, stop=True)
                t = cp.tile([P, 4 * NB], BF16, tag=f"M4_{kc}", name=f"M4_{kc}")
                nc.vector.tensor_copy(t, ps)
                M4_b.append(t)
            p4ps = pp.tile([4, N], F32, tag="scl", name="p4ps", bufs=1)

            # ============ edge pipeline ============
            aggT_f = [cp.tile([P, NB], F32, tag=f"aggT{h}", name=f"aggT{h}")
                      for h in range(2)]

            for g in range(NGRP):
                h1stg = [stp.tile([P, G * N], BF16, tag=f"h1stg{h}",
                                  name=f"h1stg{h}") for h in range(2)]
                msgstg = [stp.tile([P, G * N], BF16, tag=f"msgstg{h}",
                                   name=f"msgstg{h}") for h in range(2)]
                p1stg = [stp.tile([P, G * N], BF16, tag=f"p1stg{h}",
                                  name=f"p1stg{h}") for h in range(2)]

                # stage 1: h1_pre = cross(K=3 matmul) + A'col + B'
                for i in range(G):
                    iloc = G * g + i
                    l3 = l3p.tile([3, H], BF16, tag="lhsT3", name="lhsT3")
                    nc.vector.tensor_scalar(
                        out=l3, in0=w1cb3_b,
                        scalar1=posrT_f[:, ds(iloc, 1)], scalar2=-2.0,
                        op0=ALU.mult, op1=ALU.mult)
                    for h in range(2):
                        ps = pp.tile([P, N], F32, tag="s1", name="s1", bufs=1)
                        nc.tensor.matmul(ps, l3[:, ds(P * h, P)], posT_b,
                                         start=True, stop=True)
                        nc.vector.scalar_tensor_tensor(
                            out=h1stg[h][:, ds(i * N, N)], in0=ps,
                            scalar=A2loc_f[h][:, ds(iloc, 1)], in1=BTp_f[h],
                            op0=ALU.add, op1=ALU.add)
                for h in range(2):
                    nc.scalar.activation(h1stg[h], h1stg[h], ACTF.Silu)

                # stage 2: msg_pre = eW2 @ h1 (paired psum, one cast per pair)
                for pr in range(G // 2):
                    for h in range(2):
                        ps = pp.tile([P, 2, 512], F32, tag=f"mm{h}",
                                     name=f"mm{h}", bufs=1)
                        for k in range(2):
                            i = 2 * pr + k
                            nc.tensor.matmul(ps[:, k, 0:N],
                                             w2T_b[0][:, ds(P * h, P)],
                                             h1stg[0][:, ds(i * N, N)],
                                             start=True, stop=False)
                            nc.tensor.matmul(ps[:, k, 0:N],
                                             w2T_b[1][:, ds(P * h, P)],
                                             h1stg[1][:, ds(i * N, N)],
                                             start=False, stop=True)
                        nc.vector.tensor_copy(
                            msgstg[h][:, ds(pr * 2 * N, 2 * N)].rearrange(
                                "p (a b) -> p a b", a=2),
                            ps[:, :, 0:N])
                for h in range(2):
                    nc.scalar.activation(msgstg[h], msgstg[h], ACTF.Silu,
                                         bias=eb2c[h])

                # stage 3: p1 = silu(pW1@msg + pb1) straight from PSUM on ACT;
                # agg by receiver on GpSimd (accum_out free-dim sum)
                for i in range(G):
                    iloc = G * g + i
                    for h in range(2):
                        ps = pp.tile([P, N], F32, tag="p1", name="p1")
                        nc.tensor.matmul(ps, pw1T_b[0][:, ds(P * h, P)],
                                         msgstg[0][:, ds(i * N, N)],
                                         start=True, stop=False)
                        nc.tensor.matmul(ps, pw1T_b[1][:, ds(P * h, P)],
                                         msgstg[1][:, ds(i * N, N)],
                                         start=False, stop=True)
                        nc.scalar.activation(p1stg[h][:, ds(i * N, N)], ps,
                                             ACTF.Silu, bias=pb1c[h])
                        nc.vector.tensor_reduce(
                            aggT_f[h][:, ds(iloc, 1)],
                            msgstg[h][:, ds(i * N, N)], AX.X, ALU.add)

                # stage 4: psum4 += M4_i.T @ p1_i  (pb2 handled on host)
                for i in range(G):
                    iloc = G * g + i
                    first = (iloc == 0)
                    last = (iloc == NB - 1)
                    nc.tensor.matmul(p4ps, M4_b[0][:, ds(4 * iloc, 4)],
                                     p1stg[0][:, ds(i * N, N)],
                                     start=first, stop=False,
                                     skip_group_check=True)
                    nc.tensor.matmul(p4ps, M4_b[1][:, ds(4 * iloc, 4)],
                                     p1stg[1][:, ds(i * N, N)],
                                     start=False, stop=last,
                                     skip_group_check=True)

            # ============ node MLP + residual ============
            # agg := agg - msg_self, fused with the f32r cast
            aggT_r = []
            for h in range(2):
                t = cp.tile([P, NB], F32R, tag=f"aggT{h}c", name=f"aggT{h}c")
                nc.vector.tensor_sub(t, aggT_f[h], msgself_f[h])
                aggT_r.append(t)
            nh_r = []
            for h in range(2):
                ps = pp.tile([P, NB], F32, tag="s1", name="s1", bufs=1)
                nc.tensor.matmul(ps, nw1T_r[0][:, ds(P * h, P)], ndrT_r[0],
                                 start=True, stop=False)
                nc.tensor.matmul(ps, nw1T_r[1][:, ds(P * h, P)], ndrT_r[1],
                                 start=False, stop=False)
                nc.tensor.matmul(ps, nw1T_r[2][:, ds(P * h, P)], aggT_r[0],
                                 start=False, stop=False)
                nc.tensor.matmul(ps, nw1T_r[3][:, ds(P * h, P)], aggT_r[1],
                                 start=False, stop=True)
                t = cp.tile([P, NB], F32R, tag=f"nh{h}", name=f"nh{h}")
                nc.scalar.activation(t, ps, ACTF.Silu, bias=nb1c[h])
                nh_r.append(t)
            for h in range(2):
                ps = pp.tile([P, NB], F32, tag="p1", name="p1")
                nc.tensor.matmul(ps, nw2T_r[0][:, ds(P * h, P)], nh_r[0],
                                 start=True, stop=False)
                nc.tensor.matmul(ps, nw2T_r[1][:, ds(P * h, P)], nh_r[1],
                                 start=False, stop=True)
                t = cp.tile([P, NB], F32, tag=f"noutT{h}", name=f"noutT{h}")
                nc.vector.scalar_tensor_tensor(
                    out=t, in0=ps, scalar=nb2c[h], in1=ndrT_f[h],
                    op0=ALU.add, op1=ALU.add)
                dma(out=noutT_e[ds(P * h, P), :], in_=t)

            # ============ pos products out ============
            pout_f = cp.tile([4, N], F32, tag="pout", name="pout")
            nc.vector.tensor_copy(pout_f, p4ps)
            dma(out=pout_e[:, :], in_=pout_f)

    _split_waits(nc)
    return nc


_NC_CACHE = None


def _get_nc():
    global _NC_CACHE
    if _NC_CACHE is None:
        _NC_CACHE = build()
    return _NC_CACHE


def kernel(**inputs):
    f32 = np.float32
    nodes = np.ascontiguousarray(np.asarray(inputs["nodes"], f32))
    pos = np.ascontiguousarray(np.asarray(inputs["pos"], f32))
    eW1 = np.asarray(inputs["eW1"], f32)
    eW2 = np.asarray(inputs["eW2"], f32)
    pW1 = np.asarray(inputs["pW1"], f32)
    pW2 = np.asarray(inputs["pW2"], f32)
    nW1 = np.asarray(inputs["nW1"], f32)
    nW2 = np.asarray(inputs["nW2"], f32)

    def c(x):
        return np.ascontiguousarray(x.astype(f32))

    shared = {
        "ndT": c(nodes.T),
        "posT": c(pos.T),
        "w1aT": c(eW1[:, :H].T),
        "w1bT": c(eW1[:, H:2 * H].T),
        "w1c": c(eW1[:, 2 * H].reshape(1, H)),
        "w2T": c(eW2.T),
        "pw1T": c(pW1.T),
        "pw2row": c(pW2.reshape(1, H)),
        "nw1T": c(nW1.T),
        "nw2T": c(nW2.T),
        "eb1": c(np.asarray(inputs["eb1"], f32).reshape(H, 1)),
        "eb2": c(np.asarray(inputs["eb2"], f32).reshape(H, 1)),
        "pb1": c(np.asarray(inputs["pb1"], f32).reshape(H, 1)),
        "nb1": c(np.asarray(inputs["nb1"], f32).reshape(H, 1)),
        "nb2": c(np.asarray(inputs["nb2"], f32).reshape(H, 1)),
    }
    in_maps = []
    for cix in range(NCORES):
        blk = slice(NB * cix, NB * (cix + 1))
        m = dict(shared)
        m["ndrT"] = c(nodes[blk].T)
        m["posrT"] = c(pos[blk].T)
        m["posr4rows"] = c(np.concatenate(
            [pos[blk], np.ones((NB, 1), f32)], axis=1).reshape(1, 4 * NB))
        in_maps.append(m)

    res = run_bass_kernel_spmd(_get_nc(), in_maps, list(range(NCORES))).results

    new_nodes = np.concatenate(
        [res[cix]["nodes_outT"].T for cix in range(NCORES)], axis=0)

    upd = np.zeros((N, 3), np.float64)
    pos64 = pos.astype(np.float64)
    for cix in range(NCORES):
        p4 = res[cix]["pos_out"].astype(np.float64)
        upd += pos64 * p4[3][:, None] - p4[0:3].T
    # pb2 enters every scale entry; its pos contribution is linear and exact:
    # sum_{i != j} (p_j - p_i) * pb2 = pb2 * (N * p_j - sum_i p_i)
    pb2 = float(np.asarray(inputs["pb2"]).reshape(-1)[0])
    upd += pb2 * (N * pos64 - pos64.sum(axis=0, keepdims=True))
    new_pos = (pos64 + upd).astype(f32)
    return new_nodes, new_pos


# revision 16
# speedup vs baseline: 1.5930x; 1.3401x over previous
"""EGNN layer (fully-connected graph, N=384, H=256) on 8 TRN2 NeuronCores.

Sharding: receivers are split 48 per core. Each core computes, for its 48
receivers i and all 384 senders j (self-edge included, corrected exactly):

  h1(j,i)  = silu( eW1a@n_i + eW1b@n_j + w1c*radial(i,j) + b1 )
  msg(j,i) = silu( eW2@h1 + b2 )
  agg(i)   = sum_j msg(j,i) - msg(i,i)
  nodes'   = node MLP + residual (for the 48 local nodes)
  p1(j,i)  = silu( pW1@msg + pb1 );  psum4 += (pW2*[p_i|1]).T @ p1
  pos update: sum_i (p_j - p_i)*scale = p_j*colsum - P^T S (clip never binds;
  pb2's linear contribution is added exactly on the host)

The radial term is folded into the tensor engine:
  w1c_k * radial(i,j) = w1c_k q_i (receiver bias via scalar op) + w1c_k q_j
  (folded into the sender tables B') - 2 w1c_k <p_i, p_j> (K=3 matmul).

Hot-path matmuls (per-edge stages) run in bf16 (fp32-accumulated); the
one-time precompute (sender tables, receiver biases, node MLP, pos
products) runs in float32r. Only [256,48] nodes and [4,384] pos products
leave each core; the host does the final concat / 8-way reduction.
"""

import sys

sys.path.insert(0, "/opt/trn_rl_repo")

import numpy as np

import concourse.bass as bass
import concourse.mybir as mybir
import concourse.tile as tile
from concourse.bass import ds
from concourse.bass_utils import run_bass_kernel_spmd

F32 = mybir.dt.float32
F32R = mybir.dt.float32r
BF16 = mybir.dt.bfloat16
ACTF = mybir.ActivationFunctionType
ALU = mybir.AluOpType
AX = mybir.AxisListType

N = 384          # nodes
H = 256          # hidden
NCORES = 8
NB = N // NCORES  # receivers per core (48)
G = 8            # receivers per ACT batch group
NGRP = NB // G
P = 128          # partitions


def _split_waits(nc, max_waits=1):
    """walrus in this container allows 1 inline sync-wait per instruction;
    move extra waits onto same-engine NoOp carriers inserted just before."""
    n = 0
    for f in nc.m.functions:
        for blk in f.blocks:
            out = []
            for inst in blk.instructions:
                si = getattr(inst, "sync_info", None)
                if si is not None and si.on_wait and len(si.on_wait) > max_waits:
                    waits = list(si.on_wait)
                    extra, keep = waits[:-max_waits], waits[-max_waits:]
                    for j, w in enumerate(extra):
                        d = mybir.InstNoOp(
                            name=f"{inst.name}-wsplit{j}", ins=[], outs=[])
                        d.engine = inst.engine
                        d.sync_info = mybir.SyncInfo(on_wait=[w], on_update=[])
                        out.append(d)
                    inst.sync_info = mybir.SyncInfo(
                        on_wait=keep, on_update=list(si.on_update or []))
                    n += 1
                out.append(inst)
            blk.instructions = out
    return n


def build():
    nc = bass.Bass()
    dp = nc.declare_dram_parameter

    # ---- inputs (per-core; host marshals layouts) ----
    ndT_e = dp("ndT", [H, N], F32, isOutput=False)        # nodes.T (replicated)
    posT_e = dp("posT", [3, N], F32, isOutput=False)      # pos.T
    ndrT_e = dp("ndrT", [H, NB], F32, isOutput=False)     # local nodes.T
    posrT_e = dp("posrT", [3, NB], F32, isOutput=False)   # local pos.T
    w1aT_e = dp("w1aT", [H, H], F32, isOutput=False)      # eW1[:, :256].T
    w1bT_e = dp("w1bT", [H, H], F32, isOutput=False)      # eW1[:, 256:512].T
    w1c_e = dp("w1c", [1, H], F32, isOutput=False)        # eW1[:, 512]
    w2T_e = dp("w2T", [H, H], F32, isOutput=False)        # eW2.T
    pw1T_e = dp("pw1T", [H, H], F32, isOutput=False)      # pW1.T
    pw2r_e = dp("pw2row", [1, H], F32, isOutput=False)    # pW2 row
    pr4_e = dp("posr4rows", [1, 4 * NB], F32, isOutput=False)  # [p_i|1] rows
    nw1T_e = dp("nw1T", [2 * H, H], F32, isOutput=False)  # nW1.T
    nw2T_e = dp("nw2T", [H, H], F32, isOutput=False)      # nW2.T
    eb1_e = dp("eb1", [H, 1], F32, isOutput=False)
    eb2_e = dp("eb2", [H, 1], F32, isOutput=False)
    pb1_e = dp("pb1", [H, 1], F32, isOutput=False)
    nb1_e = dp("nb1", [H, 1], F32, isOutput=False)
    nb2_e = dp("nb2", [H, 1], F32, isOutput=False)

    # ---- outputs ----
    noutT_e = dp("nodes_outT", [H, NB], F32, isOutput=True)
    pout_e = dp("pos_out", [4, N], F32, isOutput=True)

    with tile.TileContext(nc) as tc:
        with tc.tile_pool(name="const", bufs=1) as cp, \
             tc.tile_pool(name="stage", bufs=2) as stp, \
             tc.tile_pool(name="l3p", bufs=3) as l3p, \
             tc.tile_pool(name="psum", bufs=2, space="PSUM") as pp:

            dma = nc.sync.dma_start

            # ============ loads ============
            ndT_f = [cp.tile([P, N], F32, tag=f"ndT{h}", name=f"ndT{h}")
                     for h in range(2)]
            for h in range(2):
                dma(out=ndT_f[h], in_=ndT_e[ds(P * h, P), :])
            posT_f = cp.tile([3, N], F32, tag="posT", name="posT")
            dma(out=posT_f, in_=posT_e[:, :])
            ndrT_f = [cp.tile([P, NB], F32, tag=f"ndrT{h}", name=f"ndrT{h}")
                      for h in range(2)]
            for h in range(2):
                dma(out=ndrT_f[h], in_=ndrT_e[ds(P * h, P), :])
            posrT_f = cp.tile([3, NB], F32, tag="posrT", name="posrT")
            dma(out=posrT_f, in_=posrT_e[:, :])

            def load_w(ext, rows, name):
                nkc = rows // P
                tiles = [cp.tile([P, ext.shape[1]], F32, tag=f"{name}{kc}",
                                 name=f"{name}{kc}") for kc in range(nkc)]
                for kc in range(nkc):
                    dma(out=tiles[kc], in_=ext[ds(P * kc, P), :])
                return tiles

            w1aT_f = load_w(w1aT_e, H, "w1aT")
            w1bT_f = load_w(w1bT_e, H, "w1bT")
            w2T_f = load_w(w2T_e, H, "w2T")
            pw1T_f = load_w(pw1T_e, H, "pw1T")
            nw1T_f = load_w(nw1T_e, 2 * H, "nw1T")
            nw2T_f = load_w(nw2T_e, H, "nw2T")
            pw2r_f = cp.tile([1, H], F32, tag="pw2row", name="pw2row")
            dma(out=pw2r_f, in_=pw2r_e[:, :])
            pr4_f = cp.tile([1, 4 * NB], F32, tag="pr4", name="pr4")
            dma(out=pr4_f, in_=pr4_e[:, :])
            w1c_f = cp.tile([1, H], F32, tag="w1c", name="w1c")
            dma(out=w1c_f, in_=w1c_e[:, :])

            def load_bias(ext, name, rows=H):
                tiles = [cp.tile([P, 1], F32, tag=f"{name}{h}",
                                 name=f"{name}{h}") for h in range(rows // P)]
                for h in range(rows // P):
                    dma(out=tiles[h], in_=ext[ds(P * h, P), :])
                return tiles

            eb1c = load_bias(eb1_e, "eb1")
            eb2c = load_bias(eb2_e, "eb2")
            pb1c = load_bias(pb1_e, "pb1")
            nb1c = load_bias(nb1_e, "nb1")
            nb2c = load_bias(nb2_e, "nb2")

            # ===== typed working copies (f32r for precompute, bf16 hot) =====
            def to_t(src_tiles, name, dt):
                out = []
                for i, s in enumerate(src_tiles):
                    t = cp.tile(list(s.shape), dt, tag=f"{name}{i}c",
                                name=f"{name}{i}c")
                    nc.vector.tensor_copy(t, s)
                    out.append(t)
                return out

            ndT_r = to_t(ndT_f, "ndT", F32R)
            ndrT_r = to_t(ndrT_f, "ndrT", F32R)
            posT_b = to_t([posT_f], "posTb", BF16)[0]
            pw2r_r = to_t([pw2r_f], "pw2row", F32R)[0]
            pr4_r = to_t([pr4_f], "pr4", F32R)[0]
            w1aT_r = to_t(w1aT_f, "w1aT", F32R)
            w1bT_r = to_t(w1bT_f, "w1bT", F32R)
            w2T_b = to_t(w2T_f, "w2T", BF16)
            pw1T_b = to_t(pw1T_f, "pw1T", BF16)
            nw1T_r = to_t(nw1T_f, "nw1T", F32R)
            nw2T_r = to_t(nw2T_f, "nw2T", F32R)
            w1c_r = to_t([w1c_f], "w1c", F32R)[0]
            w1abT_r = []
            for kc in range(2):
                t = cp.tile([P, H], F32R, tag=f"w1abT{kc}r",
                            name=f"w1abT{kc}r")
                nc.vector.tensor_add(t, w1aT_f[kc], w1bT_f[kc])
                w1abT_r.append(t)

            # ============ q = |pos|^2 rows ============
            ones31_f = cp.tile([3, 1], F32, tag="ones31", name="ones31")
            nc.vector.memset(ones31_f, 1.0)
            ones31_r = to_t([ones31_f], "ones31", F32R)[0]
            ones13_f = cp.tile([1, 3], F32, tag="ones13", name="ones13")
            nc.vector.memset(ones13_f, 1.0)
            ones13_r = to_t([ones13_f], "ones13", F32R)[0]

            sq_r = cp.tile([3, N], F32R, tag="sq", name="sq")
            nc.vector.tensor_mul(sq_r, posT_f, posT_f)
            q_ps = pp.tile([1, N], F32, tag="scl", name="scl", bufs=1)
            nc.tensor.matmul(q_ps, ones31_r, sq_r, start=True, stop=True)
            q_r = cp.tile([1, N], F32R, tag="qrow", name="qrow")
            nc.vector.tensor_copy(q_r, q_ps)

            sqloc_r = cp.tile([3, NB], F32R, tag="sqloc", name="sqloc")
            nc.vector.tensor_mul(sqloc_r, posrT_f, posrT_f)
            qloc_ps = pp.tile([1, NB], F32, tag="scl", name="scl", bufs=1)
            nc.tensor.matmul(qloc_ps, ones31_r, sqloc_r, start=True, stop=True)
            qloc_r = cp.tile([1, NB], F32R, tag="qloc", name="qloc")
            nc.vector.tensor_copy(qloc_r, qloc_ps)

            # w1c broadcast to 3 partitions (for per-receiver cross lhsT)
            w1cb3_ps = pp.tile([3, H], F32, tag="scl", name="scl", bufs=1)
            nc.tensor.matmul(w1cb3_ps, ones13_r, w1c_r, start=True, stop=True)
            w1cb3_b = cp.tile([3, H], BF16, tag="w1cb3", name="w1cb3")
            nc.vector.tensor_copy(w1cb3_b, w1cb3_ps)

            # ============ sender tables B'[k,j] = eW1b@n_j + w1c_k q_j ======
            BTp_f = []
            for h in range(2):
                ps = pp.tile([P, N], F32, tag="s1", name="s1", bufs=2)
                nc.tensor.matmul(ps, w1bT_r[0][:, ds(P * h, P)], ndT_r[0],
                                 start=True, stop=False)
                nc.tensor.matmul(ps, w1bT_r[1][:, ds(P * h, P)], ndT_r[1],
                                 start=False, stop=False)
                nc.tensor.matmul(ps, w1c_r[:, ds(P * h, P)], q_r,
                                 start=False, stop=True)
                t = cp.tile([P, N], F32, tag=f"BTp{h}", name=f"BTp{h}")
                nc.vector.tensor_copy(t, ps)
                BTp_f.append(t)

            # ==== receiver bias A'[k,i] = eW1a@n_i + b1 + w1c_k q_i (local) ====
            A2loc_f = []
            for h in range(2):
                ps = pp.tile([P, NB], F32, tag="s1", name="s1", bufs=2)
                nc.tensor.matmul(ps, w1aT_r[0][:, ds(P * h, P)], ndrT_r[0],
                                 start=True, stop=False)
                nc.tensor.matmul(ps, w1aT_r[1][:, ds(P * h, P)], ndrT_r[1],
                                 start=False, stop=False)
                nc.tensor.matmul(ps, w1c_r[:, ds(P * h, P)], qloc_r,
                                 start=False, stop=True)
                t = cp.tile([P, NB], F32, tag=f"A2loc{h}", name=f"A2loc{h}")
                nc.vector.tensor_scalar_add(t, ps, eb1c[h])
                A2loc_f.append(t)

            # ============ self messages msg(i,i) (radial = 0 exactly) ======
            h1self_b = []
            for h in range(2):
                ps = pp.tile([P, NB], F32, tag="s1", name="s1", bufs=2)
                nc.tensor.matmul(ps, w1abT_r[0][:, ds(P * h, P)], ndrT_r[0],
                                 start=True, stop=False)
                nc.tensor.matmul(ps, w1abT_r[1][:, ds(P * h, P)], ndrT_r[1],
                                 start=False, stop=True)
                t = cp.tile([P, NB], BF16, tag=f"h1self{h}", name=f"h1self{h}")
                nc.scalar.activation(t, ps, ACTF.Silu, bias=eb1c[h])
                h1self_b.append(t)
            msgself_f = []
            for h in range(2):
                ps = pp.tile([P, NB], F32, tag="p1", name="p1")
                nc.tensor.matmul(ps, w2T_b[0][:, ds(P * h, P)], h1self_b[0],
                                 start=True, stop=False)
                nc.tensor.matmul(ps, w2T_b[1][:, ds(P * h, P)], h1self_b[1],
                                 start=False, stop=True)
                t = cp.tile([P, NB], F32, tag=f"msgself{h}", name=f"msgself{h}")
                nc.scalar.activation(t, ps, ACTF.Silu, bias=eb2c[h])
                msgself_f.append(t)

            # ==== pos-update lhsT: M4[kc][k, 4i:4i+4] = pW2[k]*[p_i|1] ====
            M4_b = []
            for kc in range(2):
                ps = pp.tile([P, 4 * NB], F32, tag="scl", name="m4ps", bufs=1)
                nc.tensor.matmul(ps, pw2r_r[:, ds(P * kc, P)], pr4_r,
                                 start=True, stop=True)
                t = cp.tile([P, 4 * NB], BF16, tag=f"M4_{kc}", name=f"M4_{kc}")
                nc.vector.tensor_copy(t, ps)
                M4_b.append(t)
            p4ps = pp.tile([4, N], F32, tag="scl", name="p4ps", bufs=1)

            # ============ edge pipeline ============
            aggT_f = [cp.tile([P, NB], F32, tag=f"aggT{h}", name=f"aggT{h}")
                      for h in range(2)]

            for g in range(NGRP):
                h1stg = [stp.tile([P, G * N], BF16, tag=f"h1stg{h}",
                                  name=f"h1stg{h}") for h in range(2)]
                msgstg = [stp.tile([P, G * N], BF16, tag=f"msgstg{h}",
                                   name=f"msgstg{h}") for h in range(2)]
                p1stg = [stp.tile([P, G * N], BF16, tag=f"p1stg{h}",
                                  name=f"p1stg{h}") for h in range(2)]

                # stage 1: h1_pre = cross(K=3 matmul) + A'col + B'
                for i in range(G):
                    iloc = G * g + i
                    l3 = l3p.tile([3, H], BF16, tag="lhsT3", name="lhsT3")
                    nc.vector.tensor_scalar(
                        out=l3, in0=w1cb3_b,
                        scalar1=posrT_f[:, ds(iloc, 1)], scalar2=-2.0,
                        op0=ALU.mult, op1=ALU.mult)
                    for h in range(2):
                        ps = pp.tile([P, N], F32, tag="s1", name="s1", bufs=2)
                        nc.tensor.matmul(ps, l3[:, ds(P * h, P)], posT_b,
                                         start=True, stop=True)
                        nc.vector.scalar_tensor_tensor(
                            out=h1stg[h][:, ds(i * N, N)], in0=ps,
                            scalar=A2loc_f[h][:, ds(iloc, 1)], in1=BTp_f[h],
                            op0=ALU.add, op1=ALU.add)
                for h in range(2):
                    nc.scalar.activation(h1stg[h], h1stg[h], ACTF.Silu)

                # stage 2: msg = silu(eW2@h1 + b2) straight from PSUM on ACT;
                # accum_out computes agg = sum_j msg for free
                for i in range(G):
                    iloc = G * g + i
                    for h in range(2):
                        ps = pp.tile([P, N], F32, tag="mm", name="mm")
                        nc.tensor.matmul(ps, w2T_b[0][:, ds(P * h, P)],
                                         h1stg[0][:, ds(i * N, N)],
                                         start=True, stop=False)
                        nc.tensor.matmul(ps, w2T_b[1][:, ds(P * h, P)],
                                         h1stg[1][:, ds(i * N, N)],
                                         start=False, stop=True)
                        nc.scalar.activation(
                            msgstg[h][:, ds(i * N, N)], ps, ACTF.Silu,
                            bias=eb2c[h],
                            accum_out=aggT_f[h][:, ds(iloc, 1)])

                # stage 3: p1_pre = pW1 @ msg (DVE evacuates, silu batched)
                for i in range(G):
                    for h in range(2):
                        ps = pp.tile([P, N], F32, tag="p1", name="p1")
                        nc.tensor.matmul(ps, pw1T_b[0][:, ds(P * h, P)],
                                         msgstg[0][:, ds(i * N, N)],
                                         start=True, stop=False)
                        nc.tensor.matmul(ps, pw1T_b[1][:, ds(P * h, P)],
                                         msgstg[1][:, ds(i * N, N)],
                                         start=False, stop=True)
                        nc.vector.tensor_copy(p1stg[h][:, ds(i * N, N)], ps)
                for h in range(2):
                    nc.scalar.activation(p1stg[h], p1stg[h], ACTF.Silu,
                                         bias=pb1c[h])

                # stage 4: psum4 += M4_i.T @ p1_i  (pb2 handled on host)
                for i in range(G):
                    iloc = G * g + i
                    first = (iloc == 0)
                    last = (iloc == NB - 1)
                    nc.tensor.matmul(p4ps, M4_b[0][:, ds(4 * iloc, 4)],
                                     p1stg[0][:, ds(i * N, N)],
                                     start=first, stop=False,
                                     skip_group_check=True)
                    nc.tensor.matmul(p4ps, M4_b[1][:, ds(4 * iloc, 4)],
                                     p1stg[1][:, ds(i * N, N)],
                                     start=False, stop=last,
                                     skip_group_check=True)

            # ============ node MLP + residual ============
            # agg := agg - msg_self, fused with the f32r cast
            aggT_r = []
            for h in range(2):
                t = cp.tile([P, NB], F32R, tag=f"aggT{h}c", name=f"aggT{h}c")
                nc.vector.tensor_sub(t, aggT_f[h], msgself_f[h])
                aggT_r.append(t)
            nh_r = []
            for h in range(2):
                ps = pp.tile([P, NB], F32, tag="s1", name="s1", bufs=2)
                nc.tensor.matmul(ps, nw1T_r[0][:, ds(P * h, P)], ndrT_r[0],
                                 start=True, stop=False)
                nc.tensor.matmul(ps, nw1T_r[1][:, ds(P * h, P)], ndrT_r[1],
                                 start=False, stop=False)
                nc.tensor.matmul(ps, nw1T_r[2][:, ds(P * h, P)], aggT_r[0],
                                 start=False, stop=False)
                nc.tensor.matmul(ps, nw1T_r[3][:, ds(P * h, P)], aggT_r[1],
                                 start=False, stop=True)
                t = cp.tile([P, NB], F32R, tag=f"nh{h}", name=f"nh{h}")
                nc.scalar.activation(t, ps, ACTF.Silu, bias=nb1c[h])
                nh_r.append(t)
            for h in range(2):
                ps = pp.tile([P, NB], F32, tag="p1", name="p1")
                nc.tensor.matmul(ps, nw2T_r[0][:, ds(P * h, P)], nh_r[0],
                                 start=True, stop=False)
                nc.tensor.matmul(ps, nw2T_r[1][:, ds(P * h, P)], nh_r[1],
                                 start=False, stop=True)
                t = cp.tile([P, NB], F32, tag=f"noutT{h}", name=f"noutT{h}")
                nc.vector.scalar_tensor_tensor(
                    out=t, in0=ps, scalar=nb2c[h], in1=ndrT_f[h],
                    op0=ALU.add, op1=ALU.add)
                dma(out=noutT_e[ds(P * h, P), :], in_=t)

            # ============ pos products out ============
            pout_f = cp.tile([4, N], F32, tag="pout", name="pout")
            nc.vector.tensor_copy(pout_f, p4ps)
            dma(out=pout_e[:, :], in_=pout_f)

    _split_waits(nc)
    return nc


_NC_CACHE = None


def _get_nc():
    global _NC_CACHE
    if _NC_CACHE is None:
        _NC_CACHE = build()
    return _NC_CACHE


def kernel(**inputs):
    f32 = np.float32
    nodes = np.ascontiguousarray(np.asarray(inputs["nodes"], f32))
    pos = np.ascontiguousarray(np.asarray(inputs["pos"], f32))
    eW1 = np.asarray(inputs["eW1"], f32)
    eW2 = np.asarray(inputs["eW2"], f32)
    pW1 = np.asarray(inputs["pW1"], f32)
    pW2 = np.asarray(inputs["pW2"], f32)
    nW1 = np.asarray(inputs["nW1"], f32)
    nW2 = np.asarray(inputs["nW2"], f32)

    def c(x):
        return np.ascontiguousarray(x.astype(f32))

    shared = {
        "ndT": c(nodes.T),
        "posT": c(pos.T),
        "w1aT": c(eW1[:, :H].T),
        "w1bT": c(eW1[:, H:2 * H].T),
        "w1c": c(eW1[:, 2 * H].reshape(1, H)),
        "w2T": c(eW2.T),
        "pw1T": c(pW1.T),
        "pw2row": c(pW2.reshape(1, H)),
        "nw1T": c(nW1.T),
        "nw2T": c(nW2.T),
        "eb1": c(np.asarray(inputs["eb1"], f32).reshape(H, 1)),
        "eb2": c(np.asarray(inputs["eb2"], f32).reshape(H, 1)),
        "pb1": c(np.asarray(inputs["pb1"], f32).reshape(H, 1)),
        "nb1": c(np.asarray(inputs["nb1"], f32).reshape(H, 1)),
        "nb2": c(np.asarray(inputs["nb2"], f32).reshape(H, 1)),
    }
    in_maps = []
    for cix in range(NCORES):
        blk = slice(NB * cix, NB * (cix + 1))
        m = dict(shared)
        m["ndrT"] = c(nodes[blk].T)
        m["posrT"] = c(pos[blk].T)
        m["posr4rows"] = c(np.concatenate(
            [pos[blk], np.ones((NB, 1), f32)], axis=1).reshape(1, 4 * NB))
        in_maps.append(m)

    res = run_bass_kernel_spmd(_get_nc(), in_maps, list(range(NCORES))).results

    new_nodes = np.concatenate(
        [res[cix]["nodes_outT"].T for cix in range(NCORES)], axis=0)

    upd = np.zeros((N, 3), np.float64)
    pos64 = pos.astype(np.float64)
    for cix in range(NCORES):
        p4 = res[cix]["pos_out"].astype(np.float64)
        upd += pos64 * p4[3][:, None] - p4[0:3].T
    # pb2 enters every scale entry; its pos contribution is linear and exact:
    # sum_{i != j} (p_j - p_i) * pb2 = pb2 * (N * p_j - sum_i p_i)
    pb2 = float(np.asarray(inputs["pb2"]).reshape(-1)[0])
    upd += pb2 * (N * pos64 - pos64.sum(axis=0, keepdims=True))
    new_pos = (pos64 + upd).astype(f32)
    return new_nodes, new_pos
